# revision 5
# baseline (speedup 1.0000x reference)
"""GCLSTM cell (Chebyshev K=3 GCN-gated LSTM) on 8 Trainium2 NeuronCores.

Sharding: nodes are partitioned contiguously across the 8 cores (12500 each).
Each core owns its node rows of X/H/C and the edges *incoming* to its nodes
(partitioned by destination column). Host pre-normalizes edge weights
(sym Laplacian scaling, cached graph preprocessing) and sorts/pads each
device's edges by (destination tile, source chunk).

Device pipeline per core:
  prop1: Tx1_i = L_hat @ H   via bulk dma_gather of H rows (4 SWDGE queues)
         + one-hot scatter matmuls on TensorE (PSUM accumulation per col tile)
  AllGather Tx1 shards -> Tx1_full (on-chip collective)
  prop2: 2 * L_hat @ Tx1 (weights pre-doubled), produced transposed;
         Tx2_T = psum - H_T
  dense: G[node, 512] = X@Wx + H@Cw0 + Tx1@Cw1 + Tx2@Cw2 + bias (4 gates)
  LSTM pointwise: C' = sig(F)*C + sig(I)*tanh(Tc); H' = sig(O)*tanh(C')

diag term of L_hat is exactly 0 (lambda_max = 2), so prop is pure scatter.
"""
import numpy as np

N = 100000
D = 128
NCORES = 8
NPC = N // NCORES            # 12500 nodes per core
TILES = (NPC + 127) // 128   # 98
NPAD = TILES * 128           # 12544
SRC_CHUNK = 32768            # int16 index limit for dma_gather
NSC = (N + SRC_CHUNK - 1) // SRC_CHUNK  # 4 source chunks

_CACHE = {}


def _host_prep(X, edge_index, edge_weight, H, C, W, b, conv_W, conv_b):
    row = np.asarray(edge_index[0], dtype=np.int64)
    col = np.asarray(edge_index[1], dtype=np.int64)
    ew = np.asarray(edge_weight, dtype=np.float32)

    deg = np.bincount(row, weights=ew.astype(np.float64), minlength=N)
    deg = deg.astype(np.float32)
    dinv = np.where(deg > 0, deg ** -0.5, 0.0).astype(np.float32)
    w = -(dinv[row] * ew * dinv[col])  # 2/lambda_max == 1

    dev = col // NPC
    per_dev = []
    # First pass: bucket counts per (device, tile, src_chunk)
    counts = np.zeros((NCORES, TILES, NSC), dtype=np.int64)
    order = np.argsort(dev * (TILES * NSC) +
                       ((col % NPC) // 128) * NSC + (row // SRC_CHUNK),
                       kind="stable")
    row_s, col_s, w_s, dev_s = row[order], col[order], w[order], dev[order]
    colloc_s = col_s % NPC
    tile_s = colloc_s // 128
    sc_s = row_s // SRC_CHUNK
    np.add.at(counts, (dev_s, tile_s, sc_s), 1)

    # Uniform chunk capacity per src chunk (same program on all cores)
    cap = np.zeros(NSC, dtype=np.int64)
    for s in range(NSC):
        cap[s] = int(np.ceil(counts[:, :, s].max() / 128))
    cap = np.maximum(cap, 1)
    ntot = int(cap.sum()) * 128          # padded edges per tile
    nchunks = int(cap.sum())             # 128-edge chunks per tile

    # slot base offset of (tile, src_chunk) within a device's padded edge list
    sc_base = np.concatenate([[0], np.cumsum(cap)[:-1]]) * 128

    idx16 = np.zeros((NCORES, TILES * ntot), dtype=np.int16)
    coloff = np.zeros((NCORES, TILES * ntot), dtype=np.float32)
    wpad = np.zeros((NCORES, TILES * ntot), dtype=np.float32)

    # position of each edge within its (dev, tile, sc) bucket
    key = dev_s * (TILES * NSC) + tile_s * NSC + sc_s
    # edges are sorted by key; rank within bucket:
    diff = np.empty(len(key), dtype=np.int64)
    diff[0] = 0
    same = key[1:] == key[:-1]
    runstart = np.zeros(len(key), dtype=np.int64)
    idxs = np.arange(len(key))
    starts = np.concatenate([[0], idxs[1:][~same]])
    runid = np.cumsum(np.concatenate([[0], (~same).astype(np.int64)]))
    rank = idxs - starts[runid]

    slot = tile_s * ntot + sc_base[sc_s] + rank
    flat_dev = dev_s
    idx16[flat_dev, slot] = (row_s % SRC_CHUNK).astype(np.int16)
    coloff[flat_dev, slot] = (colloc_s % 128).astype(np.float32)
    wpad[flat_dev, slot] = w_s

    # SBUF layouts:
    #  - dma_gather idx: idx i -> partition i%16, column i//16; replicate x8
    #  - per-chunk scalars (coloff, w): lane e -> partition e, column chunk
    ncols_idx = TILES * ntot // 16
    nchunk_tot = TILES * nchunks
    for d in range(NCORES):
        a = idx16[d].reshape(-1, 16).T            # [16, ncols_idx]
        ii = np.tile(a, (8, 1))                   # replicate to 128 partitions
        co = coloff[d].reshape(nchunk_tot, 128).T  # [128, nchunk_tot]
        ww = wpad[d].reshape(nchunk_tot, 128).T
        per_dev.append((ii, co, ww))

    # fused dense weights: rhs blocks [128f, 512gc] for X, H(Tx0), Tx1, Tx2
    Wb = np.zeros((4, D, 4 * D), dtype=np.float32)
    for g in range(4):
        Wb[0][:, g * D:(g + 1) * D] = W[g]
        Wb[1][:, g * D:(g + 1) * D] = conv_W[g, 0]
        Wb[2][:, g * D:(g + 1) * D] = conv_W[g, 1]
        Wb[3][:, g * D:(g + 1) * D] = conv_W[g, 2]
    bias = np.concatenate([b[g] + conv_b[g] for g in range(4)]).astype(np.float32)
    biasb = np.tile(bias[None, :], (128, 1))

    Xp = np.zeros((NCORES, NPAD, D), np.float32)
    Hp = np.zeros((NCORES, NPAD, D), np.float32)
    Cp = np.zeros((NCORES, NPAD, D), np.float32)
    Xs = np.asarray(X, np.float32).reshape(NCORES, NPC, D)
    Hs = np.asarray(H, np.float32).reshape(NCORES, NPC, D)
    Cs = np.asarray(C, np.float32).reshape(NCORES, NPC, D)
    Xp[:, :NPC] = Xs
    Hp[:, :NPC] = Hs
    Cp[:, :NPC] = Cs

    Hfull = np.asarray(H, np.float32)
    hc = []
    for s in range(NSC):
        hc.append(np.ascontiguousarray(Hfull[s * SRC_CHUNK:(s + 1) * SRC_CHUNK]))

    in_maps = []
    for d in range(NCORES):
        ii, co, ww = per_dev[d]
        m = {
            "Xp": Xp[d], "Hown": Hp[d], "Cp": Cp[d],
            "idx": np.ascontiguousarray(ii),
            "coloff": np.ascontiguousarray(co),
            "w1": np.ascontiguousarray(ww),
            "w2": np.ascontiguousarray(2.0 * ww),
            "Wb": Wb.reshape(4 * D, 4 * D),
            "biasb": biasb,
        }
        for s in range(NSC):
            m[f"Hc{s}"] = hc[s]
        in_maps.append(m)

    meta = dict(cap=tuple(int(c) for c in cap), ntot=ntot, nchunks=nchunks,
                ncols_idx=ncols_idx, nchunk_tot=nchunk_tot)
    return in_maps, meta


def _build_program(meta):
    import concourse.bass as bass
    import concourse.bacc as bacc
    import concourse.tile as tile
    from concourse import mybir
    from concourse.masks import make_identity

    cap = meta["cap"]
    ntot = meta["ntot"]
    nchunks = meta["nchunks"]
    ncols_idx = meta["ncols_idx"]
    nchunk_tot = meta["nchunk_tot"]
    capmax = max(cap)
    f32 = mybir.dt.float32

    nc = bacc.Bacc("TRN2", target_bir_lowering=False, debug=False,
                   num_devices=NCORES, num_swdge_queues=4)

    Hc = [nc.dram_tensor(f"Hc{s}", [min(SRC_CHUNK, N - s * SRC_CHUNK), D], f32,
                         kind="ExternalInput") for s in range(NSC)]
    Xp = nc.dram_tensor("Xp", [NPAD, D], f32, kind="ExternalInput")
    Hown = nc.dram_tensor("Hown", [NPAD, D], f32, kind="ExternalInput")
    Cp = nc.dram_tensor("Cp", [NPAD, D], f32, kind="ExternalInput")
    IDX = nc.dram_tensor("idx", [128, ncols_idx], mybir.dt.int16,
                         kind="ExternalInput")
    COL = nc.dram_tensor("coloff", [128, nchunk_tot], f32, kind="ExternalInput")
    W1 = nc.dram_tensor("w1", [128, nchunk_tot], f32, kind="ExternalInput")
    W2 = nc.dram_tensor("w2", [128, nchunk_tot], f32, kind="ExternalInput")
    WB = nc.dram_tensor("Wb", [4 * D, 4 * D], f32, kind="ExternalInput")
    BIASB = nc.dram_tensor("biasb", [128, 4 * D], f32, kind="ExternalInput")
    OUT = nc.dram_tensor("OUT", [NPAD, D], f32, kind="ExternalOutput")

    cc_in = nc.dram_tensor("cc_in", [NPC, D], f32)
    cc_out = nc.dram_tensor("cc_out", [N, D], f32, addr_space="Shared")

    qn = [0]

    def next_q():
        q = qn[0] % 4
        qn[0] += 1
        return q

    with tile.TileContext(nc) as tc:
        import contextlib
        ctx = contextlib.ExitStack()
        with ctx:
            const = ctx.enter_context(tc.tile_pool(name="const", bufs=1))
            gp = ctx.enter_context(tc.tile_pool(name="g", bufs=6))
            sp = ctx.enter_context(tc.tile_pool(name="selw", bufs=6))
            ldp = ctx.enter_context(tc.tile_pool(name="ld", bufs=3))
            tp = ctx.enter_context(tc.tile_pool(name="tt", bufs=3))
            outp = ctx.enter_context(tc.tile_pool(name="outp", bufs=3))
            ps_a = ctx.enter_context(tc.tile_pool(name="ps_a", bufs=2, space="PSUM"))
            ps_t = ctx.enter_context(tc.tile_pool(name="ps_t", bufs=2, space="PSUM"))
            ps_g = ctx.enter_context(tc.tile_pool(name="ps_g", bufs=2, space="PSUM"))

            # --- resident constants -----------------------------------------
            idx_sb = const.tile([128, ncols_idx], mybir.dt.int16)
            nc.sync.dma_start(out=idx_sb[:], in_=IDX[:])
            col_sb = const.tile([128, nchunk_tot], f32)
            nc.sync.dma_start(out=col_sb[:], in_=COL[:])
            w1_sb = const.tile([128, nchunk_tot], f32)
            nc.sync.dma_start(out=w1_sb[:], in_=W1[:])
            w2_sb = const.tile([128, nchunk_tot], f32)
            nc.sync.dma_start(out=w2_sb[:], in_=W2[:])
            wb_sb = [const.tile([128, 4 * D], f32, tag=f"wb{i}", name=f"wb{i}")
                     for i in range(4)]
            for i in range(4):
                nc.sync.dma_start(out=wb_sb[i][:], in_=WB[i * 128:(i + 1) * 128, :])
            biasb_sb = const.tile([128, 4 * D], f32)
            nc.sync.dma_start(out=biasb_sb[:], in_=BIASB[:])
            ident = const.tile([128, 128], f32)
            make_identity(nc, ident[:])
            iota_i = const.tile([128, 128], mybir.dt.int32)
            nc.gpsimd.iota(iota_i[:], pattern=[[1, 128]], base=0,
                           channel_multiplier=0)
            iota_f = const.tile([128, 128], f32)
            nc.vector.tensor_copy(out=iota_f[:], in_=iota_i[:])

            idx_cols_per_tile = ntot // 16

            def scatter_tile(t, src_tensors, w_sb, transposed):
                """Accumulate one col tile's scatter into a PSUM tile."""
                ps = ps_a.tile([128, 128], f32, tag="scat")
                ch = 0
                for s in range(NSC):
                    g = gp.tile([128, capmax, 128], f32, tag="g")
                    icol0 = t * idx_cols_per_tile + int(
                        sum(cap[:s])) * 8
                    nc.gpsimd.dma_gather(
                        out_ap=g[:, :cap[s], :],
                        in_ap=src_tensors[s][:],
                        idxs_ap=idx_sb[:, icol0:icol0 + cap[s] * 8],
                        num_idxs=cap[s] * 128,
                        num_idxs_reg=cap[s] * 128,
                        elem_size=D,
                        queue_num=next_q(),
                    )
                    for k in range(cap[s]):
                        j = t * nchunks + ch
                        selw = sp.tile([128, 128], f32, tag="selw")
                        nc.vector.tensor_scalar(
                            out=selw[:],
                            in0=iota_f[:],
                            scalar1=col_sb[:, j:j + 1],
                            scalar2=w_sb[:, j:j + 1],
                            op0=mybir.AluOpType.is_equal,
                            op1=mybir.AluOpType.mult,
                        )
                        if transposed:
                            nc.tensor.matmul(ps[:], lhsT=g[:, k, :], rhs=selw[:],
                                             start=(ch == 0),
                                             stop=(ch == nchunks - 1))
                        else:
                            nc.tensor.matmul(ps[:], lhsT=selw[:], rhs=g[:, k, :],
                                             start=(ch == 0),
                                             stop=(ch == nchunks - 1))
                        ch += 1
                return ps

            # --- phase A: prop1 --------------------------------------------
            for t in range(TILES):
                ps = scatter_tile(t, Hc, w1_sb, transposed=False)
                tx1 = outp.tile([128, 128], f32, tag="tx1")
                nc.vector.tensor_copy(out=tx1[:], in_=ps[:])
                rows = min(128, NPC - t * 128)
                nc.sync.dma_start(out=cc_in[t * 128:t * 128 + rows, :],
                                  in_=tx1[:rows, :])

            # --- phase B: AllGather ----------------------------------------
            nc.gpsimd.collective_compute(
                "AllGather",
                mybir.AluOpType.bypass,
                replica_groups=[list(range(NCORES))],
                ins=[cc_in[:]],
                outs=[cc_out[:]],
            )

            Tc = [cc_out[s * SRC_CHUNK:min(N, (s + 1) * SRC_CHUNK), :]
                  for s in range(NSC)]

            def transpose_to(sb_tile, src_tile):
                pst = ps_t.tile([128, 128], f32, tag="tr")
                nc.tensor.transpose(out=pst[:], in_=src_tile[:], identity=ident[:])
                nc.vector.tensor_copy(out=sb_tile[:], in_=pst[:])

            # --- phase C: prop2 + dense + LSTM -----------------------------
            for t in range(TILES):
                ps2 = scatter_tile(t, Tc, w2_sb, transposed=True)  # [f x n] 2*L@Tx1

                xt = ldp.tile([128, 128], f32, tag="xt")
                nc.sync.dma_start(out=xt[:], in_=Xp[t * 128:(t + 1) * 128, :])
                ht = ldp.tile([128, 128], f32, tag="ht")
                nc.sync.dma_start(out=ht[:], in_=Hown[t * 128:(t + 1) * 128, :])
                ct = ldp.tile([128, 128], f32, tag="ct")
                nc.sync.dma_start(out=ct[:], in_=Cp[t * 128:(t + 1) * 128, :])
                # reload own Tx1 tile from the device-local collective input
                rows = min(128, NPC - t * 128)
                t1t = ldp.tile([128, 128], f32, tag="t1t")
                nc.sync.dma_start(out=t1t[:rows, :],
                                  in_=cc_in[t * 128:t * 128 + rows, :])

                xT = tp.tile([128, 128], f32, tag="xT")
                transpose_to(xT, xt)
                hT = tp.tile([128, 128], f32, tag="hT")
                transpose_to(hT, ht)
                t1T = tp.tile([128, 128], f32, tag="t1T")
                transpose_to(t1T, t1t)
                t2T = tp.tile([128, 128], f32, tag="t2T")
                nc.vector.tensor_tensor(out=t2T[:], in0=ps2[:], in1=hT[:],
                                        op=mybir.AluOpType.subtract)

                gps = ps_g.tile([128, 4 * D], f32, tag="G")
                nc.tensor.matmul(gps[:], lhsT=xT[:], rhs=wb_sb[0][:],
                                 start=True, stop=False)
                nc.tensor.matmul(gps[:], lhsT=hT[:], rhs=wb_sb[1][:],
                                 start=False, stop=False)
                nc.tensor.matmul(gps[:], lhsT=t1T[:], rhs=wb_sb[2][:],
                                 start=False, stop=False)
                nc.tensor.matmul(gps[:], lhsT=t2T[:], rhs=wb_sb[3][:],
                                 start=False, stop=True)

                gs = outp.tile([128, 4 * D], f32, tag="gs")
                nc.vector.tensor_tensor(out=gs[:], in0=gps[:], in1=biasb_sb[:],
                                        op=mybir.AluOpType.add)
                act = outp.tile([128, 4 * D], f32, tag="act")
                AF = mybir.ActivationFunctionType
                nc.scalar.activation(out=act[:, 0:128], in_=gs[:, 0:128],
                                     func=AF.Sigmoid)
                nc.scalar.activation(out=act[:, 128:256], in_=gs[:, 128:256],
                                     func=AF.Sigmoid)
                nc.scalar.activation(out=act[:, 256:384], in_=gs[:, 256:384],
                                     func=AF.Tanh)
                nc.scalar.activation(out=act[:, 384:512], in_=gs[:, 384:512],
                                     func=AF.Sigmoid)

                fc = outp.tile([128, 128], f32, tag="fc")
                nc.vector.tensor_tensor(out=fc[:], in0=act[:, 128:256], in1=ct[:],
                                        op=mybir.AluOpType.mult)
                it = outp.tile([128, 128], f32, tag="it")
                nc.vector.tensor_tensor(out=it[:], in0=act[:, 0:128],
                                        in1=act[:, 256:384],
                                        op=mybir.AluOpType.mult)
                cn = outp.tile([128, 128], f32, tag="cn")
                nc.vector.tensor_tensor(out=cn[:], in0=fc[:], in1=it[:],
                                        op=mybir.AluOpType.add)
                tc_t = outp.tile([128, 128], f32, tag="tc")
                nc.scalar.activation(out=tc_t[:], in_=cn[:], func=AF.Tanh)
                hn = outp.tile([128, 128], f32, tag="hn")
                nc.vector.tensor_tensor(out=hn[:], in0=act[:, 384:512],
                                        in1=tc_t[:], op=mybir.AluOpType.mult)
                nc.sync.dma_start(out=OUT[t * 128:(t + 1) * 128, :], in_=hn[:])

    nc.compile()
    return nc


def kernel(X, edge_index, edge_weight, H, C, W, b, conv_W, conv_b):
    from concourse.bass_utils import run_bass_kernel_spmd

    in_maps, meta = _host_prep(X, edge_index, edge_weight, H, C, W, b,
                               conv_W, conv_b)
    key = (meta["cap"],)
    if key not in _CACHE:
        _CACHE[key] = _build_program(meta)
    nc = _CACHE[key]

    res = run_bass_kernel_spmd(nc, in_maps, list(range(NCORES)))
    out = np.empty((N, D), np.float32)
    for d in range(NCORES):
        out[d * NPC:(d + 1) * NPC] = res.results[d]["OUT"][:NPC]
    return out


# revision 6
# speedup vs baseline: 9.9553x; 9.9553x over previous
"""GCLSTM cell (Chebyshev K=3 GCN-gated LSTM) on 8 Trainium2 NeuronCores.

Sharding: nodes are partitioned contiguously across the 8 cores (12500 each).
Each core owns its node rows of X/H/C and the edges *incoming* to its nodes
(partitioned by destination column). Host pre-normalizes edge weights
(sym Laplacian scaling, cached graph preprocessing) and sorts/pads each
device's edges by (destination tile, source chunk).

Device pipeline per core:
  prop1: Tx1_i = L_hat @ H   via bulk dma_gather of H rows (4 SWDGE queues)
         + one-hot scatter matmuls on TensorE (PSUM accumulation per col tile)
  AllGather Tx1 shards -> Tx1_full (on-chip collective)
  prop2: 2 * L_hat @ Tx1 (weights pre-doubled), produced transposed;
         Tx2_T = psum - H_T
  dense: G[node, 512] = X@Wx + H@Cw0 + Tx1@Cw1 + Tx2@Cw2 + bias (4 gates)
  LSTM pointwise: C' = sig(F)*C + sig(I)*tanh(Tc); H' = sig(O)*tanh(C')

diag term of L_hat is exactly 0 (lambda_max = 2), so prop is pure scatter.
"""
import numpy as np

N = 100000
D = 128
NCORES = 8
NPC = N // NCORES            # 12500 nodes per core
TILES = (NPC + 127) // 128   # 98
NPAD = TILES * 128           # 12544
SRC_CHUNK = 32768            # int16 index limit for dma_gather
NSC = (N + SRC_CHUNK - 1) // SRC_CHUNK  # 4 source chunks

_CACHE = {}


def _host_prep(X, edge_index, edge_weight, H, C, W, b, conv_W, conv_b):
    row = np.asarray(edge_index[0], dtype=np.int64)
    col = np.asarray(edge_index[1], dtype=np.int64)
    ew = np.asarray(edge_weight, dtype=np.float32)

    deg = np.bincount(row, weights=ew.astype(np.float64), minlength=N)
    deg = deg.astype(np.float32)
    dinv = np.where(deg > 0, deg ** -0.5, 0.0).astype(np.float32)
    w = -(dinv[row] * ew * dinv[col])  # 2/lambda_max == 1

    dev = col // NPC
    per_dev = []
    # First pass: bucket counts per (device, tile, src_chunk)
    counts = np.zeros((NCORES, TILES, NSC), dtype=np.int64)
    order = np.argsort(dev * (TILES * NSC) +
                       ((col % NPC) // 128) * NSC + (row // SRC_CHUNK),
                       kind="stable")
    row_s, col_s, w_s, dev_s = row[order], col[order], w[order], dev[order]
    colloc_s = col_s % NPC
    tile_s = colloc_s // 128
    sc_s = row_s // SRC_CHUNK
    np.add.at(counts, (dev_s, tile_s, sc_s), 1)

    # Uniform chunk capacity per src chunk (same program on all cores)
    cap = np.zeros(NSC, dtype=np.int64)
    for s in range(NSC):
        cap[s] = int(np.ceil(counts[:, :, s].max() / 128))
    cap = np.maximum(cap, 1)
    ntot = int(cap.sum()) * 128          # padded edges per tile
    nchunks = int(cap.sum())             # 128-edge chunks per tile

    # slot base offset of (tile, src_chunk) within a device's padded edge list
    sc_base = np.concatenate([[0], np.cumsum(cap)[:-1]]) * 128

    idx16 = np.zeros((NCORES, TILES * ntot), dtype=np.int16)
    coloff = np.zeros((NCORES, TILES * ntot), dtype=np.float32)
    wpad = np.zeros((NCORES, TILES * ntot), dtype=np.float32)

    # position of each edge within its (dev, tile, sc) bucket
    key = dev_s * (TILES * NSC) + tile_s * NSC + sc_s
    # edges are sorted by key; rank within bucket:
    diff = np.empty(len(key), dtype=np.int64)
    diff[0] = 0
    same = key[1:] == key[:-1]
    runstart = np.zeros(len(key), dtype=np.int64)
    idxs = np.arange(len(key))
    starts = np.concatenate([[0], idxs[1:][~same]])
    runid = np.cumsum(np.concatenate([[0], (~same).astype(np.int64)]))
    rank = idxs - starts[runid]

    slot = tile_s * ntot + sc_base[sc_s] + rank
    flat_dev = dev_s
    idx16[flat_dev, slot] = (row_s % SRC_CHUNK).astype(np.int16)
    coloff[flat_dev, slot] = (colloc_s % 128).astype(np.float32)
    wpad[flat_dev, slot] = w_s

    # SBUF layouts:
    #  - dma_gather idx: idx i -> partition i%16, column i//16; replicate x8
    #  - per-chunk scalars (coloff, w): lane e -> partition e, column chunk
    ncols_idx = TILES * ntot // 16
    nchunk_tot = TILES * nchunks
    for d in range(NCORES):
        a = idx16[d].reshape(-1, 16).T            # [16, ncols_idx]
        ii = np.tile(a, (8, 1))                   # replicate to 128 partitions
        co = coloff[d].reshape(nchunk_tot, 128).T  # [128, nchunk_tot]
        ww = wpad[d].reshape(nchunk_tot, 128).T
        per_dev.append((ii, co, ww))

    # fused dense weights: rhs blocks [128f, 512gc] for X, H(Tx0), Tx1, Tx2
    Wb = np.zeros((4, D, 4 * D), dtype=np.float32)
    for g in range(4):
        Wb[0][:, g * D:(g + 1) * D] = W[g]
        Wb[1][:, g * D:(g + 1) * D] = conv_W[g, 0]
        Wb[2][:, g * D:(g + 1) * D] = conv_W[g, 1]
        Wb[3][:, g * D:(g + 1) * D] = conv_W[g, 2]
    bias = np.concatenate([b[g] + conv_b[g] for g in range(4)]).astype(np.float32)
    biasb = np.tile(bias[None, :], (128, 1))

    Xp = np.zeros((NCORES, NPAD, D), np.float32)
    Hp = np.zeros((NCORES, NPAD, D), np.float32)
    Cp = np.zeros((NCORES, NPAD, D), np.float32)
    Xs = np.asarray(X, np.float32).reshape(NCORES, NPC, D)
    Hs = np.asarray(H, np.float32).reshape(NCORES, NPC, D)
    Cs = np.asarray(C, np.float32).reshape(NCORES, NPC, D)
    Xp[:, :NPC] = Xs
    Hp[:, :NPC] = Hs
    Cp[:, :NPC] = Cs

    Hfull = np.asarray(H, np.float32)
    hc = []
    for s in range(NSC):
        hc.append(np.ascontiguousarray(Hfull[s * SRC_CHUNK:(s + 1) * SRC_CHUNK]))

    in_maps = []
    for d in range(NCORES):
        ii, co, ww = per_dev[d]
        m = {
            "Xp": Xp[d], "Hown": Hp[d], "Cp": Cp[d],
            "idx": np.ascontiguousarray(ii),
            "coloff": np.ascontiguousarray(co),
            "w1": np.ascontiguousarray(ww),
            "w2": np.ascontiguousarray(2.0 * ww),
            "Wb": Wb.reshape(4 * D, 4 * D),
            "biasb": biasb,
        }
        for s in range(NSC):
            m[f"Hc{s}"] = hc[s]
        in_maps.append(m)

    meta = dict(cap=tuple(int(c) for c in cap), ntot=ntot, nchunks=nchunks,
                ncols_idx=ncols_idx, nchunk_tot=nchunk_tot)
    return in_maps, meta


def _build_program(meta):
    import concourse.bass as bass
    import concourse.bacc as bacc
    import concourse.tile as tile
    from concourse import mybir
    from concourse.masks import make_identity

    cap = meta["cap"]
    ntot = meta["ntot"]
    nchunks = meta["nchunks"]
    ncols_idx = meta["ncols_idx"]
    nchunk_tot = meta["nchunk_tot"]
    capmax = max(cap)
    f32 = mybir.dt.float32

    nc = bacc.Bacc("TRN2", target_bir_lowering=False, debug=False,
                   num_devices=NCORES, num_swdge_queues=4)

    Hc = [nc.dram_tensor(f"Hc{s}", [min(SRC_CHUNK, N - s * SRC_CHUNK), D], f32,
                         kind="ExternalInput") for s in range(NSC)]
    Xp = nc.dram_tensor("Xp", [NPAD, D], f32, kind="ExternalInput")
    Hown = nc.dram_tensor("Hown", [NPAD, D], f32, kind="ExternalInput")
    Cp = nc.dram_tensor("Cp", [NPAD, D], f32, kind="ExternalInput")
    IDX = nc.dram_tensor("idx", [128, ncols_idx], mybir.dt.int16,
                         kind="ExternalInput")
    COL = nc.dram_tensor("coloff", [128, nchunk_tot], f32, kind="ExternalInput")
    W1 = nc.dram_tensor("w1", [128, nchunk_tot], f32, kind="ExternalInput")
    W2 = nc.dram_tensor("w2", [128, nchunk_tot], f32, kind="ExternalInput")
    WB = nc.dram_tensor("Wb", [4 * D, 4 * D], f32, kind="ExternalInput")
    BIASB = nc.dram_tensor("biasb", [128, 4 * D], f32, kind="ExternalInput")
    OUT = nc.dram_tensor("OUT", [NPAD, D], f32, kind="ExternalOutput")

    cc_in = nc.dram_tensor("cc_in", [NPC, D], f32)
    cc_out = nc.dram_tensor("cc_out", [N, D], f32, addr_space="Shared")

    qn = [0]

    def next_q():
        q = qn[0] % 4
        qn[0] += 1
        return q

    with tile.TileContext(nc) as tc:
        import contextlib
        ctx = contextlib.ExitStack()
        with ctx:
            const = ctx.enter_context(tc.tile_pool(name="const", bufs=1))
            gp = ctx.enter_context(tc.tile_pool(name="g", bufs=10))
            sp = ctx.enter_context(tc.tile_pool(name="selw", bufs=8))
            ldp = ctx.enter_context(tc.tile_pool(name="ld", bufs=3))
            tp = ctx.enter_context(tc.tile_pool(name="tt", bufs=3))
            outp = ctx.enter_context(tc.tile_pool(name="outp", bufs=3))
            ps_a = ctx.enter_context(tc.tile_pool(name="ps_a", bufs=3, space="PSUM"))
            ps_t = ctx.enter_context(tc.tile_pool(name="ps_t", bufs=2, space="PSUM"))
            ps_g = ctx.enter_context(tc.tile_pool(name="ps_g", bufs=2, space="PSUM"))

            # --- resident constants -----------------------------------------
            idx_sb = const.tile([128, ncols_idx], mybir.dt.int16)
            nc.sync.dma_start(out=idx_sb[:], in_=IDX[:])
            col_sb = const.tile([128, nchunk_tot], f32)
            nc.sync.dma_start(out=col_sb[:], in_=COL[:])
            w1_sb = const.tile([128, nchunk_tot], f32)
            nc.sync.dma_start(out=w1_sb[:], in_=W1[:])
            w2_sb = const.tile([128, nchunk_tot], f32)
            nc.sync.dma_start(out=w2_sb[:], in_=W2[:])
            wb_sb = [const.tile([128, 4 * D], f32, tag=f"wb{i}", name=f"wb{i}")
                     for i in range(4)]
            for i in range(4):
                nc.sync.dma_start(out=wb_sb[i][:], in_=WB[i * 128:(i + 1) * 128, :])
            biasb_sb = const.tile([128, 4 * D], f32)
            nc.sync.dma_start(out=biasb_sb[:], in_=BIASB[:])
            ident = const.tile([128, 128], f32)
            make_identity(nc, ident[:])
            iota_i = const.tile([128, 128], mybir.dt.int32)
            nc.gpsimd.iota(iota_i[:], pattern=[[1, 128]], base=0,
                           channel_multiplier=0)
            iota_f = const.tile([128, 128], f32)
            nc.vector.tensor_copy(out=iota_f[:], in_=iota_i[:])

            idx_cols_per_tile = ntot // 16

            def scatter_tile(t, src_tensors, w_sb, transposed):
                """Accumulate one col tile's scatter into a PSUM tile."""
                ps = ps_a.tile([128, 128], f32, tag="scat")
                ch = 0
                for s in range(NSC):
                    g = gp.tile([128, capmax, 128], f32, tag="g")
                    icol0 = t * idx_cols_per_tile + int(
                        sum(cap[:s])) * 8
                    nc.gpsimd.dma_gather(
                        out_ap=g[:, :cap[s], :],
                        in_ap=src_tensors[s][:],
                        idxs_ap=idx_sb[:, icol0:icol0 + cap[s] * 8],
                        num_idxs=cap[s] * 128,
                        num_idxs_reg=cap[s] * 128,
                        elem_size=D,
                        queue_num=next_q(),
                    )
                    for k in range(cap[s]):
                        j = t * nchunks + ch
                        selw = sp.tile([128, 128], f32, tag="selw")
                        nc.vector.tensor_scalar(
                            out=selw[:],
                            in0=iota_f[:],
                            scalar1=col_sb[:, j:j + 1],
                            scalar2=w_sb[:, j:j + 1],
                            op0=mybir.AluOpType.is_equal,
                            op1=mybir.AluOpType.mult,
                        )
                        if transposed:
                            nc.tensor.matmul(ps[:], lhsT=g[:, k, :], rhs=selw[:],
                                             start=(ch == 0),
                                             stop=(ch == nchunks - 1))
                        else:
                            nc.tensor.matmul(ps[:], lhsT=selw[:], rhs=g[:, k, :],
                                             start=(ch == 0),
                                             stop=(ch == nchunks - 1))
                        ch += 1
                return ps

            # --- phase A: prop1 --------------------------------------------
            for t in range(TILES):
                ps = scatter_tile(t, Hc, w1_sb, transposed=False)
                tx1 = outp.tile([128, 128], f32, tag="tx1")
                nc.vector.tensor_copy(out=tx1[:], in_=ps[:])
                rows = min(128, NPC - t * 128)
                nc.sync.dma_start(out=cc_in[t * 128:t * 128 + rows, :],
                                  in_=tx1[:rows, :])

            # --- phase B: AllGather ----------------------------------------
            nc.gpsimd.collective_compute(
                "AllGather",
                mybir.AluOpType.bypass,
                replica_groups=[list(range(NCORES))],
                ins=[cc_in[:]],
                outs=[cc_out[:]],
            )

            Tc = [cc_out[s * SRC_CHUNK:min(N, (s + 1) * SRC_CHUNK), :]
                  for s in range(NSC)]

            def transpose_to(sb_tile, src_tile):
                pst = ps_t.tile([128, 128], f32, tag="tr")
                nc.tensor.transpose(out=pst[:], in_=src_tile[:], identity=ident[:])
                nc.vector.tensor_copy(out=sb_tile[:], in_=pst[:])

            # --- phase C: prop2 + dense + LSTM -----------------------------
            for t in range(TILES):
                ps2 = scatter_tile(t, Tc, w2_sb, transposed=True)  # [f x n] 2*L@Tx1

                xt = ldp.tile([128, 128], f32, tag="xt")
                nc.sync.dma_start(out=xt[:], in_=Xp[t * 128:(t + 1) * 128, :])
                ht = ldp.tile([128, 128], f32, tag="ht")
                nc.sync.dma_start(out=ht[:], in_=Hown[t * 128:(t + 1) * 128, :])
                ct = ldp.tile([128, 128], f32, tag="ct")
                nc.sync.dma_start(out=ct[:], in_=Cp[t * 128:(t + 1) * 128, :])
                # reload own Tx1 tile from the device-local collective input
                rows = min(128, NPC - t * 128)
                t1t = ldp.tile([128, 128], f32, tag="t1t")
                nc.sync.dma_start(out=t1t[:rows, :],
                                  in_=cc_in[t * 128:t * 128 + rows, :])

                xT = tp.tile([128, 128], f32, tag="xT")
                transpose_to(xT, xt)
                hT = tp.tile([128, 128], f32, tag="hT")
                transpose_to(hT, ht)
                t1T = tp.tile([128, 128], f32, tag="t1T")
                transpose_to(t1T, t1t)
                t2T = tp.tile([128, 128], f32, tag="t2T")
                nc.vector.tensor_tensor(out=t2T[:], in0=ps2[:], in1=hT[:],
                                        op=mybir.AluOpType.subtract)

                gps = ps_g.tile([128, 4 * D], f32, tag="G")
                nc.tensor.matmul(gps[:], lhsT=xT[:], rhs=wb_sb[0][:],
                                 start=True, stop=False)
                nc.tensor.matmul(gps[:], lhsT=hT[:], rhs=wb_sb[1][:],
                                 start=False, stop=False)
                nc.tensor.matmul(gps[:], lhsT=t1T[:], rhs=wb_sb[2][:],
                                 start=False, stop=False)
                nc.tensor.matmul(gps[:], lhsT=t2T[:], rhs=wb_sb[3][:],
                                 start=False, stop=True)

                gs = outp.tile([128, 4 * D], f32, tag="gs")
                nc.vector.tensor_tensor(out=gs[:], in0=gps[:], in1=biasb_sb[:],
                                        op=mybir.AluOpType.add)
                act = outp.tile([128, 4 * D], f32, tag="act")
                AF = mybir.ActivationFunctionType
                nc.scalar.activation(out=act[:, 0:128], in_=gs[:, 0:128],
                                     func=AF.Sigmoid)
                nc.scalar.activation(out=act[:, 128:256], in_=gs[:, 128:256],
                                     func=AF.Sigmoid)
                nc.scalar.activation(out=act[:, 256:384], in_=gs[:, 256:384],
                                     func=AF.Tanh)
                nc.scalar.activation(out=act[:, 384:512], in_=gs[:, 384:512],
                                     func=AF.Sigmoid)

                fc = outp.tile([128, 128], f32, tag="fc")
                nc.vector.tensor_tensor(out=fc[:], in0=act[:, 128:256], in1=ct[:],
                                        op=mybir.AluOpType.mult)
                it = outp.tile([128, 128], f32, tag="it")
                nc.vector.tensor_tensor(out=it[:], in0=act[:, 0:128],
                                        in1=act[:, 256:384],
                                        op=mybir.AluOpType.mult)
                cn = outp.tile([128, 128], f32, tag="cn")
                nc.vector.tensor_tensor(out=cn[:], in0=fc[:], in1=it[:],
                                        op=mybir.AluOpType.add)
                tc_t = outp.tile([128, 128], f32, tag="tc")
                nc.scalar.activation(out=tc_t[:], in_=cn[:], func=AF.Tanh)
                hn = outp.tile([128, 128], f32, tag="hn")
                nc.vector.tensor_tensor(out=hn[:], in0=act[:, 384:512],
                                        in1=tc_t[:], op=mybir.AluOpType.mult)
                nc.sync.dma_start(out=OUT[t * 128:(t + 1) * 128, :], in_=hn[:])

    nc.compile()
    return nc


def kernel(X, edge_index, edge_weight, H, C, W, b, conv_W, conv_b):
    from concourse.bass_utils import run_bass_kernel_spmd

    in_maps, meta = _host_prep(X, edge_index, edge_weight, H, C, W, b,
                               conv_W, conv_b)
    key = (meta["cap"],)
    if key not in _CACHE:
        _CACHE[key] = _build_program(meta)
    nc = _CACHE[key]

    res = run_bass_kernel_spmd(nc, in_maps, list(range(NCORES)))
    out = np.empty((N, D), np.float32)
    for d in range(NCORES):
        out[d * NPC:(d + 1) * NPC] = res.results[d]["OUT"][:NPC]
    return out
